# revision 2
# baseline (speedup 1.0000x reference)
"""Trainium2 Bass kernel for nn_CustomLoss (CrossEntropy + binary-remap BCE).

loss = mean_i[ logsumexp(pred_i) - pred_i[t_i] ]
     + 100 * mean_i[ 1{ LUT[argmax(pred_i)] != LUT[t_i] } ]

with LUT = [0,0,1,1,1,1,1,1,0,0]  (so LUT[j] = 1 iff 2 <= j <= 7).

Sharding: data-parallel over the batch axis across 8 NeuronCores. Each core
streams its row slice once and produces 3 per-partition partial sums
(sum of log-sum-exp, sum of gathered logits, count of binary mismatches);
the host combines the 8x128x3 partials into the scalar loss.

Per-core device pipeline (all engines busy, memory-bound target):
  DMA   : pred tiles [128, W*10] f32 (contiguous rows), aux gs tiles [128, W]
  ACT   : E = exp(pred);  Ln(s) with per-partition accumulate
  GPSIMD: per-row sum of E via a strided add tree (10 -> 5 -> 2+1 -> 1)
  DVE   : per-row max of mid-6 / outer-4 classes (argmax-group test),
          custom fused op GATHER_EQ_SUM (one-hot gather + accumulate),
          custom fused op MISMATCH_XOR_SUM
The gather index + binary target are packed on the host into one f32 aux
tensor gs = +-(10*w + t), sign = binary target (negative zero for w=0,t=0).
"""

import numpy as np

# ---------------------------------------------------------------- constants
N = 2_000_000
C = 10
N_CORES = 8
P = 128
W = 326                      # rows per partition per tile
TILES = 6
ROWS_PER_TILE = P * W        # 41,728
ROWS_CORE_PAD = ROWS_PER_TILE * TILES   # 250,368
ROWS_CORE = N // N_CORES     # 250,000
PAD_PER_CORE = ROWS_CORE_PAD - ROWS_CORE  # 368

_CACHE = {}


# ------------------------------------------------------- custom DVE ops
def _register_custom_ops():
    """Register the two fused DVE ops (idempotent)."""
    import concourse.dve_ops as dve_ops
    from concourse.dve_spec import (
        Spec, Src0, Src1, Zero, select, eq, lower, AluOp, Idx, Bin,
        maxx,
    )
    from concourse.dve_uop import DveOpSpec
    from concourse.dve_table_gen import dve_ver_for

    def _get(name):
        for op in dve_ops.OPS:
            if op.name == name:
                return op
        return None

    def _register(name, spec):
        existing = _get(name)
        if existing is not None:
            return existing
        opcode = dve_ops._CUSTOM_DVE_ROW_BASE + len(dve_ops.OPS)
        assert opcode < 0x20, "custom DVE opcode rows exhausted"
        from concourse.dve_ops import has_src1
        shas = {}
        for ver in ("v3", "v4"):
            uops = lower(spec, ver=ver)
            tmp = DveOpSpec(name=name, opcode=opcode, uops=uops,
                            rd1_en=has_src1(spec))
            shas[ver] = tmp.sha(ver)
        op = dve_ops.DveOp(name, spec, subdim=False, uops_sha=shas)
        dve_ops.OPS.append(op)
        dve_ops._SUB_OPCODE_FOR_NAME[name] = opcode
        dve_ops.CUSTOM_DVE_SPECS[name] = spec
        return op

    # GATHER_EQ_SUM: out[k] = in0[k] if k == |in1[k]| else 0
    #                accum  = sum_k out[k]
    # in1 is the per-row gather index (10*w + t) broadcast along the class
    # dim, signed by the binary target; |.| strips the sign.
    def _gather_ref(in0, in1, s0, s1, imm2):
        p = in0.shape[0]
        f0 = np.asarray(in0, np.float32).reshape(p, -1)
        f1 = np.abs(np.asarray(in1, np.float32).reshape(p, -1))
        idx = np.arange(f0.shape[1], dtype=np.float32)[None, :]
        out = np.where(idx == f1, f0, np.float32(0.0))
        acc = out.sum(axis=1, dtype=np.float64).astype(np.float32)[:, None]
        return out.reshape(in0.shape), acc

    gather_spec = Spec(
        body=select(eq(Idx, maxx(Src1, Zero - Src1)), Src0, Zero),
        accum=AluOp.ADD,
        accum_init=Zero,
        reference=_gather_ref,
    )
    gop = _register("GATHER_EQ_SUM_ANT2", gather_spec)

    # MISMATCH_XOR_SUM: out[k] = (in0[k] != 0) XOR (in1[k] > 0)
    #                   accum  = sum_k out[k]
    # in0 = predicted binary label (0/1), in1 = signed gs (sign = target
    # binary label).
    def _mm_ref(in0, in1, s0, s1, imm2):
        p = in0.shape[0]
        a = np.asarray(in0, np.float32).reshape(p, -1) != 0
        b = np.asarray(in1, np.float32).reshape(p, -1) > 0
        out = (a ^ b).astype(np.float32)
        acc = out.sum(axis=1, dtype=np.float64).astype(np.float32)[:, None]
        return out.reshape(in0.shape), acc

    mm_spec = Spec(
        body=Bin(AluOp.LOGICAL_XOR, Src0, Src1 > Zero),
        accum=AluOp.ADD,
        accum_init=Zero,
        reference=_mm_ref,
    )
    mop = _register("MISMATCH_XOR_SUM_ANT2", mm_spec)
    return gop, mop


# ------------------------------------------------------------- device build
def _build_nc(tiles=TILES, w=W):
    import concourse.bass as bass
    import concourse.tile as tile
    from concourse import bacc, mybir

    gop, mop = _register_custom_ops()
    f32 = mybir.dt.float32
    A = mybir.ActivationFunctionType
    X = mybir.AxisListType.X
    alu = mybir.AluOpType

    nc = bacc.Bacc("TRN2", target_bir_lowering=False, debug=False,
                   num_devices=N_CORES)
    pred_d = nc.dram_tensor("pred", [tiles, P, w * C], f32,
                            kind="ExternalInput").ap()
    gs_d = nc.dram_tensor("gs", [tiles, P, w], f32,
                          kind="ExternalInput").ap()
    out_d = nc.dram_tensor("out", [P, 3], f32, kind="ExternalOutput").ap()

    with tile.TileContext(nc) as tc:
        with (
            tc.tile_pool(name="io", bufs=2) as io,
            tc.tile_pool(name="ep", bufs=2) as ep,
            tc.tile_pool(name="zp", bufs=2) as zp,
            tc.tile_pool(name="mp", bufs=2) as mp,
            tc.tile_pool(name="tp", bufs=1) as tp,
            tc.tile_pool(name="accp", bufs=1) as accp,
        ):
            acc_lg = accp.tile([P, tiles], f32)
            acc_g = accp.tile([P, tiles], f32)
            acc_mm = accp.tile([P, tiles], f32)
            trash = tp.tile([P, w * C], f32)
            trash2 = tp.tile([P, w], f32)

            for i in range(tiles):
                pt = io.tile([P, w * C], f32, tag="pred")
                nc.sync.dma_start(pt[:], pred_d[i])
                gt = io.tile([P, w], f32, tag="gs")
                nc.sync.dma_start(gt[:], gs_d[i])

                # ---- CE path: exp on ACT, row-sum tree on GPSIMD, ln on ACT
                et = ep.tile([P, w * C], f32, tag="E")
                nc.scalar.activation(et[:], pt[:], A.Exp)

                e3 = et[:].rearrange("p (w c) -> p w c", c=C)
                z1 = zp.tile([P, w, 5], f32, tag="z1")
                nc.gpsimd.tensor_tensor(z1[:], e3[:, :, 0:5], e3[:, :, 5:10],
                                        op=alu.add)
                z2 = zp.tile([P, w, 2], f32, tag="z2")
                nc.gpsimd.tensor_tensor(z2[:], z1[:, :, 0:2], z1[:, :, 2:4],
                                        op=alu.add)
                z3 = zp.tile([P, w], f32, tag="z3")
                nc.gpsimd.tensor_tensor(z3[:], z2[:, :, 0], z2[:, :, 1],
                                        op=alu.add)
                s = zp.tile([P, w], f32, tag="s")
                nc.gpsimd.tensor_tensor(s[:], z3[:], z1[:, :, 4], op=alu.add)

                lg = zp.tile([P, w], f32, tag="lg")
                nc.scalar.activation(lg[:], s[:], A.Ln,
                                     accum_out=acc_lg[:, i:i + 1])

                # ---- BCE path: group maxes on DVE
                p3 = pt[:].rearrange("p (w c) -> p w c", c=C)
                p4 = pt[:].rearrange("p (w g e) -> p w g e", g=5, e=2)
                m6 = mp.tile([P, w], f32, tag="m6")
                nc.vector.reduce_max(m6[:], p3[:, :, 2:8], axis=X)
                ma = mp.tile([P, w], f32, tag="ma")
                nc.vector.reduce_max(ma[:], p4[:, :, 0, :], axis=X)
                mb = mp.tile([P, w], f32, tag="mb")
                nc.vector.reduce_max(mb[:], p4[:, :, 4, :], axis=X)
                m4 = mp.tile([P, w], f32, tag="m4")
                nc.vector.tensor_tensor(m4[:], ma[:], mb[:], op=alu.max)
                bp = mp.tile([P, w], f32, tag="bp")
                nc.vector.tensor_tensor(bp[:], m6[:], m4[:], op=alu.is_gt)

                # ---- fused gather + mismatch accumulation
                nc.vector._custom_dve(
                    gop, out=trash[:],
                    in0=pt[:].rearrange("p (w c) -> p w c", c=C),
                    in1=gt[:].unsqueeze(2).broadcast_to([P, w, C]),
                    accum_out=acc_g[:, i:i + 1])
                nc.vector._custom_dve(
                    mop, out=trash2[:], in0=bp[:], in1=gt[:],
                    accum_out=acc_mm[:, i:i + 1])

            # ---- final per-partition reductions + store
            out_t = accp.tile([P, 3], f32)
            nc.vector.reduce_sum(out_t[:, 0:1], acc_lg[:], axis=X)
            nc.vector.reduce_sum(out_t[:, 1:2], acc_g[:], axis=X)
            nc.vector.reduce_sum(out_t[:, 2:3], acc_mm[:], axis=X)
            nc.sync.dma_start(out_d[:], out_t[:])

    nc.compile()
    return nc


def _get_nc():
    if "nc" not in _CACHE:
        _CACHE["nc"] = _build_nc()
    return _CACHE["nc"]


# ------------------------------------------------------------------- host
def _host_prep(pred, target):
    """Shard + pad inputs, build the packed gs aux tensor per core."""
    pred = np.ascontiguousarray(np.asarray(pred, dtype=np.float32))
    target = np.asarray(target).astype(np.int32)

    in_maps = []
    rows = ROWS_CORE
    for c in range(N_CORES):
        pc = pred[c * rows:(c + 1) * rows]
        tc_ = target[c * rows:(c + 1) * rows]
        if PAD_PER_CORE:
            pc = np.concatenate(
                [pc, np.zeros((PAD_PER_CORE, C), np.float32)], axis=0)
            tc_ = np.concatenate(
                [tc_, np.zeros(PAD_PER_CORE, np.int32)], axis=0)
        pc = pc.reshape(TILES, P, W * C)
        tc_ = tc_.reshape(TILES, P, W)
        w_idx = np.broadcast_to(
            np.arange(W, dtype=np.int64) * C, (TILES, P, W))
        g = (w_idx + tc_).astype(np.float32)
        bt = (tc_ >= 2) & (tc_ <= 7)
        gs = np.where(bt, g, -g).astype(np.float32)
        in_maps.append({"pred": np.ascontiguousarray(pc),
                        "gs": np.ascontiguousarray(gs)})
    return in_maps


def kernel(pred, target):
    from concourse.bass_utils import run_bass_kernel_spmd

    nc = _get_nc()
    in_maps = _host_prep(pred, target)
    res = run_bass_kernel_spmd(nc, in_maps, core_ids=list(range(N_CORES)))

    sum_lg = 0.0
    sum_g = 0.0
    sum_mm = 0.0
    for c in range(N_CORES):
        o = res.results[c]["out"].astype(np.float64)
        sum_lg += o[:, 0].sum()
        sum_g += o[:, 1].sum()
        sum_mm += o[:, 2].sum()

    # remove the padded rows' contribution: each pad row is all-zero pred
    # -> logsumexp = ln(10), gather = 0, mismatch = 0
    sum_lg -= N_CORES * PAD_PER_CORE * np.log(10.0)

    ce = (sum_lg - sum_g) / N
    bce = 100.0 * sum_mm / N
    return np.float32(ce + bce)
